# revision 46
# baseline (speedup 1.0000x reference)
"""Trainium2 Bass kernel for nn_ARTLayer (gnn_message_passing).

Math (reference):
    j(i,t) = t + (t>=i)                                    # [K, K-1] neighbor index
    alpha  = sigmoid(x@wa [i] + x@wb [j] + pf@wc + b_att)  # [K, K-1]
    msgs   = mean_t alpha * ((x@WobjT + b_obj)[j] + pf@WpairT + b_pair)
    out    = LN(x + msgs); out = LN(out + FFN(out))

Key algebraic rewrite (removes the 34-GFLOP [P,PD]x[PD,H] einsum):
    sum_t a*(pf@WpT)  = (sum_t a*pf) @ WpT               -> U[i,:] @ WpT
    sum_t a*oj[j]     = (A @ x) @ WobjT                  -> Gx[i,:] @ WoT
      with A[i,j] decomposed via lo/hi shifted views of x and a t>=i mask
    sum_t a*(b_obj+b_pair) = s_alpha[i] * bop

Sharding: rows i split across 8 cores (64 each); small tensors replicated;
host concatenates the per-core [64, 512] outputs.

Implementation notes (driven by NTFF profiles):
  - wc is folded into pf on the host (column scales, floored at fp16
    min-normal); sc becomes a pure fp16 halving-tree add-reduce and U is
    recovered exactly via W_pairT rows pre-divided by the scales.
  - 1/511 (the neighbor mean) is folded into WpT/WoT/bop on the host, and
    the t=511 pad slot is poisoned with -1e9 pre-sigmoid, so raw sigmoid
    output is used directly with no mask/scale multiplies.
  - U accumulation runs as M=4 quad matmuls (alpha quad stationary, four pf
    blocks streaming at N=512); the wanted rows sit on the block diagonal
    and are gathered by a stride-640 DRAM access pattern after a bank dump.
  - pf is re-laid-out on the host to [chunk, t, i, pd] so each tile DMA is
    one fully-contiguous 16KB-per-partition burst. One HWDGE queue (sync)
    carries the critical path in order (packed const blobs, then pf chunks);
    tail-only weights stream on the second HWDGE queue (scalar) in parallel.
  - U rows are extracted on-chip (bank copy -> per-block PE transpose ->
    free-strided gather copies); no DRAM bounce.
"""
import numpy as np

import concourse.bass as bass
import concourse.tile as tile
from concourse import bacc, mybir

F32, F16 = mybir.dt.float32, mybir.dt.float16
AX = mybir.AxisListType
OP = mybir.AluOpType
AF = mybir.ActivationFunctionType

K, D, H, PD = 512, 512, 512, 128
T = K - 1                      # 511 neighbors per row
NCORES, IPC = 8, 64            # rows per core
NCH = 4                        # t-chunks of 128 (last chunk row 127 is t=511 pad)
IB, NIB = 64, 1                # i-block within a core
EPS = 1e-5


def build_program() -> bacc.Bacc:
    nc = bacc.Bacc("TRN2", target_bir_lowering=False, debug=False)

    def inp(name, shape, dt):
        return nc.dram_tensor(name, shape, dt, kind="ExternalInput").ap()

    pf = inp("pf", [NCH, 128, IPC, PD], F16)    # [chunk, t-in-chunk, i, pd]
    # single-DMA packed critical constants (128 contiguous rows each):
    # b32: [cmat 0:512 | poison 512:640 | mask_lt 640:896 | b_att 896:904]
    # b16: [wa_t 0:512 | wb_t 512:1024 | mask_ge 1024:1280 | ones 1280:1288
    #       | xi16 1288:1800]
    b32 = inp("b32", [128, 904], F32)
    b16 = inp("b16", [128, 1800], F16)
    xlo_ch = inp("xlo_ch", [128, NCH, D], F16)  # x rows chunked [t%128, t//128]
    dxf = inp("dxf", [K, D], F16)               # x[t+1] - x[t], host computed
    xi = inp("xi", [IPC, D], F32)               # this core's rows of x
    bias5 = inp("bias5", [5, H], F32)           # [ln_g; ln_b; b1; b2; bop/511]
    WpT = inp("WpT", [PD, H], F16)              # W_pair.T / colscale / 511
    WoT = inp("WoT", [D, H], F16)               # W_obj.T / 511
    W1T = inp("W1T", [H, H], F16)
    W2T = inp("W2T", [H, H], F16)

    out_d = nc.dram_tensor("out", [IPC, H], F32, kind="ExternalOutput").ap()

    with tile.TileContext(nc) as tc:
        with (
            tc.tile_pool(name="const", bufs=1) as cpool,
            tc.tile_pool(name="pfp", bufs=4) as pfp,
            tc.tile_pool(name="scrp", bufs=3) as scrp,
            tc.tile_pool(name="smallp", bufs=4) as smallp,
            tc.tile_pool(name="postp", bufs=3) as postp,
            tc.tile_pool(name="pss", bufs=2, space="PSUM") as pss,
            tc.tile_pool(name="psflex", bufs=4, space="PSUM") as psflex,
            tc.tile_pool(name="psmp", bufs=1, space="PSUM") as psmp,
            tc.tile_pool(name="psgp", bufs=1, space="PSUM") as psgp,
        ):
            # DMA policy: one HWDGE queue (sync) carries the critical path
            # in program order (a single queue reaches ~313 GB/s here and
            # multi-queue round-robin measures WORSE); bulky tail-only
            # weights stream on the gpsimd SWDGE queue in parallel.
            def dma(out, in_):
                nc.sync.dma_start(out=out, in_=in_)

            def dma_late(out, in_):
                nc.scalar.dma_start(out=out, in_=in_)

            # ---- constants & weights to SBUF (3 packed critical DMAs) ----
            hp = tc.high_priority()
            hp.__enter__()
            b32_sb = cpool.tile([128, 904], F32)
            dma(b32_sb, b32)
            b16_sb = cpool.tile([128, 1800], F16)
            dma(b16_sb, b16)
            xlo = cpool.tile([128, NCH, D], F16)
            dma(xlo, xlo_ch)
            hp.__exit__(None, None, None)
            tc.no_sync_barrier()   # keep pf DMAs behind the critical consts
            id_sb = b32_sb[:, 0:128]
            ones_sb = b32_sb[:, 128:256]
            sh1_sb = b32_sb[:, 256:384]
            sh2_sb = b32_sb[:, 384:512]
            poison_sb = b32_sb[0:1, 512:640]
            mlt_sb = b32_sb[:, 640:896].rearrange("p (c i) -> p c i", c=NCH)
            b_att_col = b32_sb[0:IPC, 896:897]
            wa_b = b16_sb[0:IPC, 0:512]
            wb_b = b16_sb[:, 512:1024]
            mge_sb = b16_sb[:, 1024:1280].rearrange("p (c i) -> p c i", c=NCH)
            ones16_sb = b16_sb[:, 1280:1288]
            xi16_sb = b16_sb[0:IPC, 1288:1800]
            bias_sb = cpool.tile([IPC, 5, H], F32)
            dma_late(bias_sb, bias5[None, :, :].to_broadcast([IPC, 5, H]))
            gb_sb = bias_sb[:, 0, :]
            bb_sb = bias_sb[:, 1, :]
            b1_sb = bias_sb[:, 2, :]
            b2_sb = bias_sb[:, 3, :]
            bop_row = bias_sb[0:1, 4, :]
            # tail-only loads on the slow queue, in rough use order
            dx = cpool.tile([128, NCH, D], F16)
            dma_late(dx, dxf.rearrange("(c p) d -> p c d", p=128))
            WpT_sb = cpool.tile([128, H], F16)
            dma_late(WpT_sb, WpT)
            WoT_sb = cpool.tile([128, NCH, H], F16)
            dma_late(WoT_sb, WoT.rearrange("(c p) h -> p c h", p=128))
            xi_sb = cpool.tile([IPC, D], F32)
            dma_late(xi_sb, xi)
            W1T_sb = cpool.tile([128, NCH, H], F16)
            dma_late(W1T_sb, W1T.rearrange("(c p) h -> p c h", p=128))
            W2T_sb = cpool.tile([128, NCH, H], F16)
            dma_late(W2T_sb, W2T.rearrange("(c p) h -> p c h", p=128))

            eps_col = cpool.tile([IPC, 1], F32)
            nc.vector.memset(eps_col, EPS)

            # ---- sa (this core's rows) and sb (all rows) ----
            scr_sa = smallp.tile([IPC, D], F16)
            nc.vector.tensor_mul(scr_sa, xi16_sb, wa_b)
            sa_col = smallp.tile([IPC, 1], F32)
            nc.vector.tensor_reduce(sa_col, scr_sa, axis=AX.X, op=OP.add)
            nc.vector.tensor_add(sa_col, sa_col, b_att_col)
            sa_diag = smallp.tile([IPC, IPC], F32)
            nc.vector.tensor_mul(sa_diag, id_sb[0:IPC, 0:IPC],
                                 sa_col.to_broadcast([IPC, IPC]))

            sb_cols = smallp.tile([128, NCH], F32)
            for c in range(NCH):
                scr_sb = smallp.tile([128, D], F16)
                nc.vector.tensor_mul(scr_sb, xlo[:, c, :], wb_b)
                nc.vector.tensor_reduce(
                    sb_cols[:, c:c + 1], scr_sb, axis=AX.X, op=OP.add)

            # sb_hi[p, c] = sb[c*128+p+1] via shift matmuls; slot 511 stays 0
            sbhi_ps = pss.tile([128, NCH], F32, tag="ps_small")
            nc.tensor.matmul(sbhi_ps, sh1_sb, sb_cols, start=True, stop=False)
            nc.tensor.matmul(sbhi_ps[:, 0:NCH - 1], sh2_sb, sb_cols[:, 1:NCH],
                             start=False, stop=True)
            sbhi_cols = smallp.tile([128, NCH], F32)
            nc.vector.tensor_copy(sbhi_cols, sbhi_ps)

            # ---- SBJ[t, i] = sa[i] + b_att + sb_hi[t] + mask_lt*(sb_lo-sb_hi),
            #      with -1e9 poison at the t=511 pad slot ----
            sbj = cpool.tile([128, NCH, IPC], F32)
            for c in range(NCH):
                diffc = smallp.tile([128, 1], F32)
                nc.vector.tensor_tensor(
                    diffc, sb_cols[:, c:c + 1], sbhi_cols[:, c:c + 1], OP.subtract)
                diagc = smallp.tile([128, 128], F32)
                nc.vector.tensor_mul(diagc, id_sb, diffc.to_broadcast([128, 128]))
                diagb = smallp.tile([128, 128], F32)
                nc.vector.tensor_mul(
                    diagb, id_sb, sbhi_cols[:, c:c + 1].to_broadcast([128, 128]))
                ps_sbj = pss.tile([128, IPC], F32, tag="ps_small")
                nc.tensor.matmul(ps_sbj, ones_sb[0:IPC, :], sa_diag,
                                 start=True, stop=False)
                nc.tensor.matmul(ps_sbj, diagb, ones_sb[:, 0:IPC],
                                 start=False, stop=False)
                if c == NCH - 1:
                    # poison: sigmoid(-1e9) = 0 exactly, pad row drops out
                    nc.tensor.matmul(ps_sbj, poison_sb, ones_sb[0:1, 0:IPC],
                                     start=False, stop=False)
                nc.tensor.matmul(ps_sbj, diagc, mlt_sb[:, c, :],
                                 start=False, stop=True)
                nc.vector.tensor_copy(sbj[:, c, :], ps_sbj)

            # ---- main edge pass ----
            alpha_full = cpool.tile([128, NCH, IPC], F16)   # raw sigmoid out
            age_full = cpool.tile([128, NCH, IPC], F16)     # masked (t>=i) alpha
            gx_ps = psgp.tile([IPC, D], F32)                # sum_t a*x[j]
            msg_ps = psmp.tile([IPC, H], F32)
            s_ps = pss.tile([1, IPC], F32, tag="ps_small")
            # U quad rows: bank b, partition slot 32s..32s+3 holds i=16b+4s+j
            u_ps = [psflex.tile([128, 512], F32, tag="flex", name=f"u_ps{b}")
                    for b in range(4)]
            for b in range(4):
                nc.vector.memset(u_ps[b], 0.0)

            for c in range(NCH):
                pf_t = pfp.tile([128, IB, PD], F16, tag="pf_t")
                dma(pf_t, pf[c, :, :, :])
                # sc = sum_pd pf_sent (wc pre-folded): fp16 halving tree
                scr = scrp.tile([128, IB, 64], F16, tag="scr")
                nc.vector.tensor_add(scr, pf_t[:, :, 0:64], pf_t[:, :, 64:128])
                w = 32
                while w >= 2:
                    nc.vector.tensor_add(
                        scr[:, :, 0:w], scr[:, :, 0:w], scr[:, :, w:2 * w])
                    w //= 2
                sc_t = smallp.tile([128, IB], F32, tag="sc_t")
                nc.vector.tensor_add(sc_t, scr[:, :, 0], scr[:, :, 1])
                aarg = smallp.tile([128, IB], F32)
                nc.vector.tensor_add(aarg, sc_t, sbj[:, c, :])
                nc.scalar.activation(alpha_full[:, c, :], aarg, AF.Sigmoid)
                nc.vector.tensor_mul(age_full[:, c, :], alpha_full[:, c, :],
                                     mge_sb[:, c, :])
                # U quads: lhsT = 4 alpha columns, rhs = 4 pf blocks; the
                # wanted rows sit on the diagonal (gathered via DRAM AP)
                for q in range(IB // 4):
                    b, sp = divmod(q, 4)
                    nc.tensor.matmul(
                        u_ps[b][32 * sp:32 * sp + 4, :],
                        alpha_full[:, c, 4 * q:4 * q + 4],
                        pf_t[:, 4 * q:4 * q + 4, :],
                        start=(c == 0), stop=(c == NCH - 1),
                        tile_position=(0, 32 * sp))
                nc.tensor.matmul(gx_ps, alpha_full[:, c, :], xlo[:, c, :],
                                 start=(c == 0), stop=False)
                nc.tensor.matmul(s_ps, ones16_sb[:, 0:1], alpha_full[:, c, :],
                                 start=(c == 0), stop=(c == NCH - 1))

            # scheduler fence: keep every tail instruction after the loop in
            # each engine stream (strict-FIFO engines head-of-line block if
            # e.g. an LN Sqrt lands between loop sigmoids in the ACT queue)
            tc.no_sync_barrier()

            # G2 (shifted-x correction) after the loop: dx arrives on the slow
            # queue and age_full persists, so this overlaps the loop tail
            for c in range(NCH):
                nc.tensor.matmul(gx_ps, age_full[:, c, :], dx[:, c, :],
                                 start=False, stop=(c == NCH - 1))

            # ---- messages = U@WpT + Gx@WoT + s_alpha x bop ----
            s_row = smallp.tile([1, IPC], F32)
            nc.vector.tensor_copy(s_row, s_ps)

            # U reassembly on-chip: bank copy -> PE transpose of each
            # 128-col block (diagonal quad becomes free-strided columns) ->
            # tiny strided copies assemble UT directly; no DRAM bounce.
            u_sb = postp.tile([128, IPC], F16)
            for b in range(4):
                u_cp = postp.tile([128, 512], F32, tag="u_cp")
                nc.vector.tensor_copy(u_cp, u_ps[b])
                for j in range(4):
                    ptu = pss.tile([128, 128], F32, tag="ps_small")
                    nc.tensor.transpose(ptu, u_cp[:, j * 128:(j + 1) * 128],
                                        id_sb)
                    # cols {j, 32+j, 64+j, 96+j} hold U rows i=16b+4s+j
                    src_ap = ptu.rearrange("p (s q) -> p s q", q=32)[:, :, j]
                    dst_ap = u_sb.rearrange("p (r s f) -> p r s f", r=4, s=4)[
                        :, b, :, j]
                    nc.vector.tensor_copy(dst_ap, src_ap)

            gx_sb = postp.tile([IPC, D], F32)
            nc.vector.tensor_copy(gx_sb, gx_ps)
            gxT = postp.tile([128, NCH, IPC], F16)
            for c in range(NCH):
                ptg = pss.tile([128, IPC], F32, tag="ps_small")
                nc.tensor.transpose(ptg, gx_sb[:, c * 128:(c + 1) * 128],
                                    id_sb[0:IPC, 0:IPC])
                nc.vector.tensor_copy(gxT[:, c, :], ptg)

            for c in range(NCH):
                nc.tensor.matmul(msg_ps, gxT[:, c, :], WoT_sb[:, c, :],
                                 start=(c == 0), stop=False)
            nc.tensor.matmul(msg_ps, s_row, bop_row, start=False, stop=False)
            nc.tensor.matmul(msg_ps, u_sb, WpT_sb, start=False, stop=True)

            # ---- residual + LN1 ----
            def layer_norm(v):
                stats = smallp.tile([IPC, 6], F32)
                nc.vector.bn_stats(out=stats, in_=v)
                mv = smallp.tile([IPC, 2], F32)
                nc.vector.bn_aggr(out=mv, in_=stats)
                std = smallp.tile([IPC, 1], F32)
                nc.scalar.activation(std, mv[:, 1:2], AF.Sqrt, bias=eps_col)
                rstd = smallp.tile([IPC, 1], F32)
                nc.vector.reciprocal(rstd, std)
                cen = postp.tile([IPC, H], F32)
                nc.vector.tensor_scalar(cen, v, mv[:, 0:1], rstd,
                                        OP.subtract, OP.mult)
                o = postp.tile([IPC, H], F32)
                nc.vector.tensor_mul(o, cen, gb_sb)
                nc.vector.tensor_add(o, o, bb_sb)
                return o

            h_sb = postp.tile([IPC, H], F32)
            nc.vector.tensor_add(h_sb, xi_sb, msg_ps)
            out1 = layer_norm(h_sb)

            # ---- FFN ----
            def transpose_rows(v):
                vT = postp.tile([128, NCH, IPC], F16, tag="vT")
                for c in range(NCH):
                    ptt = pss.tile([128, IPC], F32, tag="ps_small")
                    nc.tensor.transpose(ptt, v[:, c * 128:(c + 1) * 128],
                                        id_sb[0:IPC, 0:IPC])
                    nc.vector.tensor_copy(vT[:, c, :], ptt)
                return vT

            o1T = transpose_rows(out1)
            o1b = postp.tile([IPC, H], F32)
            nc.vector.tensor_add(o1b, out1, b2_sb)
            f1_ps = psflex.tile([IPC, H], F32, tag="flex")
            for c in range(NCH):
                nc.tensor.matmul(f1_ps, o1T[:, c, :], W1T_sb[:, c, :],
                                 start=(c == 0), stop=(c == NCH - 1))
            f1 = postp.tile([IPC, H], F32)
            nc.vector.tensor_add(f1, f1_ps, b1_sb)
            nc.vector.tensor_scalar_max(f1, f1, 0.0)

            f1T = transpose_rows(f1)
            f2_ps = psflex.tile([IPC, H], F32, tag="flex")
            for c in range(NCH):
                nc.tensor.matmul(f2_ps, f1T[:, c, :], W2T_sb[:, c, :],
                                 start=(c == 0), stop=(c == NCH - 1))
            h2 = postp.tile([IPC, H], F32)
            nc.vector.tensor_add(h2, f2_ps, o1b)
            out2 = layer_norm(h2)

            nc.sync.dma_start(out=out_d, in_=out2)

    return nc


def _poison() -> np.ndarray:
    p = np.zeros((1, 128), np.float32)
    p[0, 127] = -1e9
    return p


def _cmat() -> np.ndarray:
    c = np.zeros((128, 4, 128), np.float32)
    c[:, 0, :] = np.eye(128)
    c[:, 1, :] = 1.0
    c[:, 2, :] = np.eye(128, k=-1)     # shift1[q, p] = (q == p+1)
    c[0, 3, 127] = 1.0                  # shift2[q, p] = (q==0)&(p==127)
    return c


def prep_in_maps(inputs) -> list[dict]:
    x = np.asarray(inputs["x"], np.float32)
    pf = np.asarray(inputs["pair_feats"], np.float32)
    W_att = np.asarray(inputs["W_att"], np.float32)
    b_att = np.asarray(inputs["b_att"], np.float32)
    W_obj = np.asarray(inputs["W_obj"], np.float32)
    b_obj = np.asarray(inputs["b_obj"], np.float32)
    W_pair = np.asarray(inputs["W_pair"], np.float32)
    b_pair = np.asarray(inputs["b_pair"], np.float32)
    ln_g = np.asarray(inputs["ln_g"], np.float32)
    ln_b = np.asarray(inputs["ln_b"], np.float32)
    W1 = np.asarray(inputs["W1"], np.float32)
    b1 = np.asarray(inputs["b1"], np.float32)
    W2 = np.asarray(inputs["W2"], np.float32)
    b2 = np.asarray(inputs["b2"], np.float32)

    wa, wb, wc = W_att[0, :D], W_att[0, D:2 * D], W_att[0, 2 * D:]
    xpad = np.concatenate([x, np.zeros((1, D), np.float32)], axis=0)

    # fold wc into pf columns; recover U via pre-divided W_pair.T rows.
    colscale = np.sign(wc) * np.maximum(np.abs(wc), 6e-5)
    colscale[colscale == 0] = 6e-5
    # 1/511 (the mean over neighbors) is folded into the three weight paths
    # that consume raw alpha: U@WpT, (A@x)@WoT, and s_alpha*bop.
    WpT2 = (W_pair.T / colscale[:, None] / T).astype(np.float16)
    WoT2 = (W_obj.T / T).astype(np.float16)
    dxf = np.diff(xpad[:K + 1], axis=0)

    b32a = np.zeros((128, 904), np.float32)
    b32a[:, 0:512] = _cmat().reshape(128, 512)
    b32a[0, 512 + 127] = -1e9
    b32a[:, 896] = b_att[0]
    b16a = np.zeros((128, 1800), np.float16)
    b16a[:, 0:512] = wa[None, :]
    b16a[:, 512:1024] = wb[None, :]
    b16a[:, 1280:1288] = 1.0
    xlo_np = np.ascontiguousarray(
        x.reshape(NCH, 128, D).transpose(1, 0, 2)).astype(np.float16)

    base = dict(
        xlo_ch=xlo_np,
        dxf=dxf.astype(np.float16),
        bias5=np.stack([ln_g, ln_b, b1, b2,
                        (b_obj + b_pair) / T]).astype(np.float32),
        WpT=np.ascontiguousarray(WpT2),
        WoT=np.ascontiguousarray(WoT2),
        W1T=np.ascontiguousarray(W1.T).astype(np.float16),
        W2T=np.ascontiguousarray(W2.T).astype(np.float16),
    )

    pfr = pf.reshape(K, T, PD)
    tgrid = np.arange(128)[:, None] + 128 * np.arange(NCH)[None, :]   # [128, NCH]

    in_maps = []
    for core in range(NCORES):
        ig = np.arange(core * IPC, (core + 1) * IPC)
        mlt = (tgrid[:, :, None] < ig[None, None, :]).astype(np.float32)
        mge = ((tgrid[:, :, None] >= ig[None, None, :])
               & (tgrid[:, :, None] <= T - 1)).astype(np.float16)
        # [chunk, t, i, pd] layout -> each tile DMA is one contiguous burst
        shard = np.zeros((NCH * 128, IPC, PD), np.float16)
        shard[:T] = (pfr[ig] * colscale[None, None, :]).transpose(1, 0, 2)
        xi = x[ig]
        cb32 = b32a.copy()
        cb32[:, 640:896] = mlt.reshape(128, NCH * IPC)
        cb16 = b16a.copy()
        cb16[:, 1024:1280] = mge.reshape(128, NCH * IPC)
        cb16[0:IPC, 1288:1800] = xi.astype(np.float16)
        m = dict(base)
        m.update(
            pf=shard.reshape(NCH, 128, IPC, PD),
            xi=xi.astype(np.float32),
            b32=cb32,
            b16=cb16,
        )
        in_maps.append(m)
    return in_maps


_COMPILED = None


def _get_program() -> bacc.Bacc:
    global _COMPILED
    if _COMPILED is None:
        nc = build_program()
        nc.compile()
        _COMPILED = nc
    return _COMPILED


TRACE = False
LAST_RESULT = None


def _install_axon_ntff_hook():
    """The container's antenv lacks axon_hooks; recreate it from trn_boot's
    ctypes implementation so trace=True can capture NTFF profiles."""
    import sys
    import types
    try:
        from antenv.axon_hooks import get_axon_ntff_profile_hook  # noqa: F401
        return
    except ImportError:
        pass
    from trn_agent_boot.trn_boot import _ntff_profile_via_ctypes
    hook = _ntff_profile_via_ctypes("/opt/axon/libaxon_pjrt.so")
    m = types.ModuleType("antenv.axon_hooks")
    m.get_axon_ntff_profile_hook = lambda: hook
    sys.modules["antenv.axon_hooks"] = m


def kernel(**inputs) -> np.ndarray:
    import concourse.bass_utils as bu
    from concourse.bass_utils import run_bass_kernel_spmd
    global LAST_RESULT
    if TRACE:
        _install_axon_ntff_hook()
        bu.upload_artifacts = lambda tmpdir: str(tmpdir)  # no bucket here
    nc = _get_program()
    in_maps = prep_in_maps(inputs)
    res = run_bass_kernel_spmd(nc, in_maps, list(range(NCORES)), trace=TRACE)
    LAST_RESULT = res
    outs = [res.results[c]["out"] for c in range(NCORES)]
    return np.concatenate(outs, axis=0).astype(np.float32)


# revision 48
# speedup vs baseline: 1.0016x; 1.0016x over previous
"""Trainium2 Bass kernel for nn_ARTLayer (gnn_message_passing).

Math (reference):
    j(i,t) = t + (t>=i)                                    # [K, K-1] neighbor index
    alpha  = sigmoid(x@wa [i] + x@wb [j] + pf@wc + b_att)  # [K, K-1]
    msgs   = mean_t alpha * ((x@WobjT + b_obj)[j] + pf@WpairT + b_pair)
    out    = LN(x + msgs); out = LN(out + FFN(out))

Key algebraic rewrite (removes the 34-GFLOP [P,PD]x[PD,H] einsum):
    sum_t a*(pf@WpT)  = (sum_t a*pf) @ WpT               -> U[i,:] @ WpT
    sum_t a*oj[j]     = (A @ x) @ WobjT                  -> Gx[i,:] @ WoT
      with A[i,j] decomposed via lo/hi shifted views of x and a t>=i mask
    sum_t a*(b_obj+b_pair) = s_alpha[i] * bop

Sharding: rows i split across 8 cores (64 each); small tensors replicated;
host concatenates the per-core [64, 512] outputs.

Implementation notes (driven by NTFF profiles):
  - wc is folded into pf on the host (column scales, floored at fp16
    min-normal); sc becomes a pure fp16 halving-tree add-reduce and U is
    recovered exactly via W_pairT rows pre-divided by the scales.
  - 1/511 (the neighbor mean) is folded into WpT/WoT/bop on the host, and
    the t=511 pad slot is poisoned with -1e9 pre-sigmoid, so raw sigmoid
    output is used directly with no mask/scale multiplies.
  - U accumulation runs as M=4 quad matmuls (alpha quad stationary, four pf
    blocks streaming at N=512); the wanted rows sit on the block diagonal
    and are gathered by a stride-640 DRAM access pattern after a bank dump.
  - pf is re-laid-out on the host to [chunk, t, i, pd] so each tile DMA is
    one fully-contiguous 16KB-per-partition burst. One HWDGE queue (sync)
    carries the critical path in order (packed const blobs, then pf chunks);
    tail-only weights stream on the second HWDGE queue (scalar) in parallel.
  - U rows are extracted on-chip (bank copy -> per-block PE transpose ->
    free-strided gather copies); no DRAM bounce.
"""
import numpy as np

import concourse.bass as bass
import concourse.tile as tile
from concourse import bacc, mybir

F32, F16 = mybir.dt.float32, mybir.dt.float16
AX = mybir.AxisListType
OP = mybir.AluOpType
AF = mybir.ActivationFunctionType

K, D, H, PD = 512, 512, 512, 128
T = K - 1                      # 511 neighbors per row
NCORES, IPC = 8, 64            # rows per core
NCH = 4                        # t-chunks of 128 (last chunk row 127 is t=511 pad)
IB, NIB = 64, 1                # i-block within a core
EPS = 1e-5


def build_program() -> bacc.Bacc:
    nc = bacc.Bacc("TRN2", target_bir_lowering=False, debug=False)

    def inp(name, shape, dt):
        return nc.dram_tensor(name, shape, dt, kind="ExternalInput").ap()

    pf = inp("pf", [NCH, 128, IPC, PD], F16)    # [chunk, t-in-chunk, i, pd]
    # single-DMA packed critical constants (128 contiguous rows each):
    # b32: [cmat 0:512 | poison 512:640 | mask_lt 640:896 | b_att 896:904]
    # b16: [wa_t 0:512 | wb_t 512:1024 | mask_ge 1024:1280 | ones 1280:1288
    #       | xi16 1288:1800]
    b32 = inp("b32", [128, 904], F32)
    b16 = inp("b16", [128, 1800], F16)
    xlo_ch = inp("xlo_ch", [128, NCH, D], F16)  # x rows chunked [t%128, t//128]
    dxf = inp("dxf", [K, D], F16)               # x[t+1] - x[t], host computed
    xi = inp("xi", [IPC, D], F32)               # this core's rows of x
    bias5 = inp("bias5", [5, H], F32)           # [ln_g; ln_b; b1; b2; bop/511]
    WpT = inp("WpT", [PD, H], F16)              # W_pair.T / colscale / 511
    WoT = inp("WoT", [D, H], F16)               # W_obj.T / 511
    W1T = inp("W1T", [H, H], F16)
    W2T = inp("W2T", [H, H], F16)

    out_d = nc.dram_tensor("out", [IPC, H], F32, kind="ExternalOutput").ap()

    with tile.TileContext(nc) as tc:
        with (
            tc.tile_pool(name="const", bufs=1) as cpool,
            tc.tile_pool(name="pfp", bufs=4) as pfp,
            tc.tile_pool(name="scrp", bufs=3) as scrp,
            tc.tile_pool(name="smallp", bufs=4) as smallp,
            tc.tile_pool(name="postp", bufs=3) as postp,
            tc.tile_pool(name="pss", bufs=2, space="PSUM") as pss,
            tc.tile_pool(name="psflex", bufs=4, space="PSUM") as psflex,
            tc.tile_pool(name="psmp", bufs=1, space="PSUM") as psmp,
            tc.tile_pool(name="psgp", bufs=1, space="PSUM") as psgp,
        ):
            # DMA policy: one HWDGE queue (sync) carries the critical path
            # in program order (a single queue reaches ~313 GB/s here and
            # multi-queue round-robin measures WORSE); bulky tail-only
            # weights stream on the gpsimd SWDGE queue in parallel.
            def dma(out, in_):
                nc.sync.dma_start(out=out, in_=in_)

            def dma_late(out, in_):
                nc.scalar.dma_start(out=out, in_=in_)

            # ---- constants & weights to SBUF (3 packed critical DMAs) ----
            hp = tc.high_priority()
            hp.__enter__()
            b32_sb = cpool.tile([128, 904], F32)
            dma(b32_sb, b32)
            b16_sb = cpool.tile([128, 1800], F16)
            dma(b16_sb, b16)
            xlo = cpool.tile([128, NCH, D], F16)
            dma(xlo, xlo_ch)
            hp.__exit__(None, None, None)
            tc.no_sync_barrier()   # keep pf DMAs behind the critical consts
            id_sb = b32_sb[:, 0:128]
            ones_sb = b32_sb[:, 128:256]
            sh1_sb = b32_sb[:, 256:384]
            sh2_sb = b32_sb[:, 384:512]
            poison_sb = b32_sb[0:1, 512:640]
            mlt_sb = b32_sb[:, 640:896].rearrange("p (c i) -> p c i", c=NCH)
            b_att_col = b32_sb[0:IPC, 896:897]
            wa_b = b16_sb[0:IPC, 0:512]
            wb_b = b16_sb[:, 512:1024]
            mge_sb = b16_sb[:, 1024:1280].rearrange("p (c i) -> p c i", c=NCH)
            ones16_sb = b16_sb[:, 1280:1288]
            xi16_sb = b16_sb[0:IPC, 1288:1800]
            bias_sb = cpool.tile([IPC, 5, H], F32)
            dma_late(bias_sb, bias5[None, :, :].to_broadcast([IPC, 5, H]))
            gb_sb = bias_sb[:, 0, :]
            bb_sb = bias_sb[:, 1, :]
            b1_sb = bias_sb[:, 2, :]
            b2_sb = bias_sb[:, 3, :]
            bop_row = bias_sb[0:1, 4, :]
            # tail-only loads on the slow queue, in rough use order
            dx = cpool.tile([128, NCH, D], F16)
            dma_late(dx, dxf.rearrange("(c p) d -> p c d", p=128))
            WpT_sb = cpool.tile([128, H], F16)
            dma_late(WpT_sb, WpT)
            WoT_sb = cpool.tile([128, NCH, H], F16)
            dma_late(WoT_sb, WoT.rearrange("(c p) h -> p c h", p=128))
            xi_sb = cpool.tile([IPC, D], F32)
            dma_late(xi_sb, xi)
            W1T_sb = cpool.tile([128, NCH, H], F16)
            dma_late(W1T_sb, W1T.rearrange("(c p) h -> p c h", p=128))
            W2T_sb = cpool.tile([128, NCH, H], F16)
            dma_late(W2T_sb, W2T.rearrange("(c p) h -> p c h", p=128))

            eps_col = cpool.tile([IPC, 1], F32)
            nc.vector.memset(eps_col, EPS)

            # ---- sa (this core's rows) and sb (all rows) ----
            scr_sa = smallp.tile([IPC, D], F16)
            nc.vector.tensor_mul(scr_sa, xi16_sb, wa_b)
            sa_col = smallp.tile([IPC, 1], F32)
            nc.vector.tensor_reduce(sa_col, scr_sa, axis=AX.X, op=OP.add)
            nc.vector.tensor_add(sa_col, sa_col, b_att_col)
            sa_diag = smallp.tile([IPC, IPC], F32)
            nc.vector.tensor_mul(sa_diag, id_sb[0:IPC, 0:IPC],
                                 sa_col.to_broadcast([IPC, IPC]))

            sb_cols = smallp.tile([128, NCH], F32)
            for c in range(NCH):
                scr_sb = smallp.tile([128, D], F16)
                nc.vector.tensor_mul(scr_sb, xlo[:, c, :], wb_b)
                nc.vector.tensor_reduce(
                    sb_cols[:, c:c + 1], scr_sb, axis=AX.X, op=OP.add)

            # sb_hi[p, c] = sb[c*128+p+1] via shift matmuls; slot 511 stays 0
            sbhi_ps = pss.tile([128, NCH], F32, tag="ps_small")
            nc.tensor.matmul(sbhi_ps, sh1_sb, sb_cols, start=True, stop=False)
            nc.tensor.matmul(sbhi_ps[:, 0:NCH - 1], sh2_sb, sb_cols[:, 1:NCH],
                             start=False, stop=True)
            sbhi_cols = smallp.tile([128, NCH], F32)
            nc.vector.tensor_copy(sbhi_cols, sbhi_ps)

            # ---- SBJ[t, i] = sa[i] + b_att + sb_hi[t] + mask_lt*(sb_lo-sb_hi),
            #      with -1e9 poison at the t=511 pad slot ----
            sbj = cpool.tile([128, NCH, IPC], F32)
            for c in range(NCH):
                diffc = smallp.tile([128, 1], F32)
                nc.vector.tensor_tensor(
                    diffc, sb_cols[:, c:c + 1], sbhi_cols[:, c:c + 1], OP.subtract)
                diagc = smallp.tile([128, 128], F32)
                nc.vector.tensor_mul(diagc, id_sb, diffc.to_broadcast([128, 128]))
                diagb = smallp.tile([128, 128], F32)
                nc.vector.tensor_mul(
                    diagb, id_sb, sbhi_cols[:, c:c + 1].to_broadcast([128, 128]))
                ps_sbj = pss.tile([128, IPC], F32, tag="ps_small")
                nc.tensor.matmul(ps_sbj, ones_sb[0:IPC, :], sa_diag,
                                 start=True, stop=False)
                nc.tensor.matmul(ps_sbj, diagb, ones_sb[:, 0:IPC],
                                 start=False, stop=False)
                if c == NCH - 1:
                    # poison: sigmoid(-1e9) = 0 exactly, pad row drops out
                    nc.tensor.matmul(ps_sbj, poison_sb, ones_sb[0:1, 0:IPC],
                                     start=False, stop=False)
                nc.tensor.matmul(ps_sbj, diagc, mlt_sb[:, c, :],
                                 start=False, stop=True)
                nc.vector.tensor_copy(sbj[:, c, :], ps_sbj)

            # ---- main edge pass ----
            alpha_full = cpool.tile([128, NCH, IPC], F16)   # raw sigmoid out
            age_full = cpool.tile([128, NCH, IPC], F16)     # masked (t>=i) alpha
            gx_ps = psgp.tile([IPC, D], F32)                # sum_t a*x[j]
            msg_ps = psmp.tile([IPC, H], F32)
            s_ps = pss.tile([1, IPC], F32, tag="ps_small")
            # U quad rows: bank b, partition slot 32s..32s+3 holds i=16b+4s+j
            u_ps = [psflex.tile([128, 512], F32, tag="flex", name=f"u_ps{b}")
                    for b in range(4)]
            for b in range(4):
                nc.vector.memset(u_ps[b], 0.0)

            for c in range(NCH):
                pf_t = pfp.tile([128, IB, PD], F16, tag="pf_t")
                dma(pf_t, pf[c, :, :, :])
                # sc = sum_pd pf_sent (wc pre-folded): fp16 halving tree
                scr = scrp.tile([128, IB, 64], F16, tag="scr")
                nc.vector.tensor_add(scr, pf_t[:, :, 0:64], pf_t[:, :, 64:128])
                w = 32
                while w >= 2:
                    nc.vector.tensor_add(
                        scr[:, :, 0:w], scr[:, :, 0:w], scr[:, :, w:2 * w])
                    w //= 2
                sc_t = smallp.tile([128, IB], F32, tag="sc_t")
                nc.vector.tensor_add(sc_t, scr[:, :, 0], scr[:, :, 1])
                aarg = smallp.tile([128, IB], F32)
                nc.vector.tensor_add(aarg, sc_t, sbj[:, c, :])
                nc.scalar.activation(alpha_full[:, c, :], aarg, AF.Sigmoid)
                nc.vector.tensor_mul(age_full[:, c, :], alpha_full[:, c, :],
                                     mge_sb[:, c, :])
                # U quads: lhsT = 4 alpha columns, rhs = 4 pf blocks; the
                # wanted rows sit on the diagonal (gathered via DRAM AP)
                for q in range(IB // 4):
                    b, sp = divmod(q, 4)
                    nc.tensor.matmul(
                        u_ps[b][32 * sp:32 * sp + 4, :],
                        alpha_full[:, c, 4 * q:4 * q + 4],
                        pf_t[:, 4 * q:4 * q + 4, :],
                        start=(c == 0), stop=(c == NCH - 1),
                        tile_position=(0, 32 * sp))
                nc.tensor.matmul(gx_ps, alpha_full[:, c, :], xlo[:, c, :],
                                 start=(c == 0), stop=False)
                nc.tensor.matmul(s_ps, ones16_sb[:, 0:1], alpha_full[:, c, :],
                                 start=(c == 0), stop=(c == NCH - 1))

            # scheduler fence: keep every tail instruction after the loop in
            # each engine stream (strict-FIFO engines head-of-line block if
            # e.g. an LN Sqrt lands between loop sigmoids in the ACT queue)
            tc.no_sync_barrier()

            # G2 (shifted-x correction) after the loop: dx arrives on the slow
            # queue and age_full persists, so this overlaps the loop tail
            for c in range(NCH):
                nc.tensor.matmul(gx_ps, age_full[:, c, :], dx[:, c, :],
                                 start=False, stop=(c == NCH - 1))

            # ---- messages = U@WpT + Gx@WoT + s_alpha x bop ----
            s_row = smallp.tile([1, IPC], F32)
            nc.vector.tensor_copy(s_row, s_ps)

            # U reassembly on-chip: bank copy -> PE transpose of each
            # 128-col block (diagonal quad becomes free-strided columns) ->
            # tiny strided copies assemble UT directly; no DRAM bounce.
            u_sb = postp.tile([128, IPC], F16)
            for b in range(4):
                u_cp = postp.tile([128, 512], F32, tag="u_cp")
                nc.vector.tensor_copy(u_cp, u_ps[b])
                for j in range(4):
                    ptu = pss.tile([128, 128], F32, tag="ps_small")
                    nc.tensor.transpose(ptu, u_cp[:, j * 128:(j + 1) * 128],
                                        id_sb)
                    # cols {j, 32+j, 64+j, 96+j} hold U rows i=16b+4s+j
                    src_ap = ptu.rearrange("p (s q) -> p s q", q=32)[:, :, j]
                    dst_ap = u_sb.rearrange("p (r s f) -> p r s f", r=4, s=4)[
                        :, b, :, j]
                    nc.vector.tensor_copy(dst_ap, src_ap)

            gx_sb = postp.tile([IPC, D], F32)
            nc.vector.tensor_copy(gx_sb, gx_ps)
            gxT = postp.tile([128, NCH, IPC], F16)
            for c in range(NCH):
                ptg = pss.tile([128, IPC], F32, tag="ps_small")
                nc.tensor.transpose(ptg, gx_sb[:, c * 128:(c + 1) * 128],
                                    id_sb[0:IPC, 0:IPC])
                nc.vector.tensor_copy(gxT[:, c, :], ptg)

            # gx-side MMs carry the accumulation-group start (their input is
            # ready first); Upp closes the group after the U extraction
            for c in range(NCH):
                nc.tensor.matmul(msg_ps, gxT[:, c, :], WoT_sb[:, c, :],
                                 start=(c == 0), stop=False)
            nc.tensor.matmul(msg_ps, s_row, bop_row, start=False, stop=False)
            nc.tensor.matmul(msg_ps, u_sb, WpT_sb, start=False, stop=True)

            # ---- residual + LN1 ----
            def layer_norm(v):
                stats = smallp.tile([IPC, 6], F32)
                nc.vector.bn_stats(out=stats, in_=v)
                mv = smallp.tile([IPC, 2], F32)
                nc.vector.bn_aggr(out=mv, in_=stats)
                std = smallp.tile([IPC, 1], F32)
                nc.scalar.activation(std, mv[:, 1:2], AF.Sqrt, bias=eps_col)
                rstd = smallp.tile([IPC, 1], F32)
                nc.vector.reciprocal(rstd, std)
                cen = postp.tile([IPC, H], F32)
                nc.vector.tensor_scalar(cen, v, mv[:, 0:1], rstd,
                                        OP.subtract, OP.mult)
                o = postp.tile([IPC, H], F32)
                nc.vector.tensor_mul(o, cen, gb_sb)
                nc.vector.tensor_add(o, o, bb_sb)
                return o

            h_sb = postp.tile([IPC, H], F32)
            nc.vector.tensor_add(h_sb, xi_sb, msg_ps)
            out1 = layer_norm(h_sb)

            # ---- FFN ----
            def transpose_rows(v):
                vT = postp.tile([128, NCH, IPC], F16, tag="vT")
                for c in range(NCH):
                    ptt = pss.tile([128, IPC], F32, tag="ps_small")
                    nc.tensor.transpose(ptt, v[:, c * 128:(c + 1) * 128],
                                        id_sb[0:IPC, 0:IPC])
                    nc.vector.tensor_copy(vT[:, c, :], ptt)
                return vT

            o1T = transpose_rows(out1)
            o1b = postp.tile([IPC, H], F32)
            nc.vector.tensor_add(o1b, out1, b2_sb)
            f1_ps = psflex.tile([IPC, H], F32, tag="flex")
            for c in range(NCH):
                nc.tensor.matmul(f1_ps, o1T[:, c, :], W1T_sb[:, c, :],
                                 start=(c == 0), stop=(c == NCH - 1))
            f1 = postp.tile([IPC, H], F32)
            nc.vector.tensor_add(f1, f1_ps, b1_sb)
            nc.vector.tensor_scalar_max(f1, f1, 0.0)

            f1T = transpose_rows(f1)
            f2_ps = psflex.tile([IPC, H], F32, tag="flex")
            for c in range(NCH):
                nc.tensor.matmul(f2_ps, f1T[:, c, :], W2T_sb[:, c, :],
                                 start=(c == 0), stop=(c == NCH - 1))
            h2 = postp.tile([IPC, H], F32)
            nc.vector.tensor_add(h2, f2_ps, o1b)
            out2 = layer_norm(h2)

            nc.sync.dma_start(out=out_d, in_=out2)

    return nc


def _poison() -> np.ndarray:
    p = np.zeros((1, 128), np.float32)
    p[0, 127] = -1e9
    return p


def _cmat() -> np.ndarray:
    c = np.zeros((128, 4, 128), np.float32)
    c[:, 0, :] = np.eye(128)
    c[:, 1, :] = 1.0
    c[:, 2, :] = np.eye(128, k=-1)     # shift1[q, p] = (q == p+1)
    c[0, 3, 127] = 1.0                  # shift2[q, p] = (q==0)&(p==127)
    return c


def prep_in_maps(inputs) -> list[dict]:
    x = np.asarray(inputs["x"], np.float32)
    pf = np.asarray(inputs["pair_feats"], np.float32)
    W_att = np.asarray(inputs["W_att"], np.float32)
    b_att = np.asarray(inputs["b_att"], np.float32)
    W_obj = np.asarray(inputs["W_obj"], np.float32)
    b_obj = np.asarray(inputs["b_obj"], np.float32)
    W_pair = np.asarray(inputs["W_pair"], np.float32)
    b_pair = np.asarray(inputs["b_pair"], np.float32)
    ln_g = np.asarray(inputs["ln_g"], np.float32)
    ln_b = np.asarray(inputs["ln_b"], np.float32)
    W1 = np.asarray(inputs["W1"], np.float32)
    b1 = np.asarray(inputs["b1"], np.float32)
    W2 = np.asarray(inputs["W2"], np.float32)
    b2 = np.asarray(inputs["b2"], np.float32)

    wa, wb, wc = W_att[0, :D], W_att[0, D:2 * D], W_att[0, 2 * D:]
    xpad = np.concatenate([x, np.zeros((1, D), np.float32)], axis=0)

    # fold wc into pf columns; recover U via pre-divided W_pair.T rows.
    colscale = np.sign(wc) * np.maximum(np.abs(wc), 6e-5)
    colscale[colscale == 0] = 6e-5
    # 1/511 (the mean over neighbors) is folded into the three weight paths
    # that consume raw alpha: U@WpT, (A@x)@WoT, and s_alpha*bop.
    WpT2 = (W_pair.T / colscale[:, None] / T).astype(np.float16)
    WoT2 = (W_obj.T / T).astype(np.float16)
    dxf = np.diff(xpad[:K + 1], axis=0)

    b32a = np.zeros((128, 904), np.float32)
    b32a[:, 0:512] = _cmat().reshape(128, 512)
    b32a[0, 512 + 127] = -1e9
    b32a[:, 896] = b_att[0]
    b16a = np.zeros((128, 1800), np.float16)
    b16a[:, 0:512] = wa[None, :]
    b16a[:, 512:1024] = wb[None, :]
    b16a[:, 1280:1288] = 1.0
    xlo_np = np.ascontiguousarray(
        x.reshape(NCH, 128, D).transpose(1, 0, 2)).astype(np.float16)

    base = dict(
        xlo_ch=xlo_np,
        dxf=dxf.astype(np.float16),
        bias5=np.stack([ln_g, ln_b, b1, b2,
                        (b_obj + b_pair) / T]).astype(np.float32),
        WpT=np.ascontiguousarray(WpT2),
        WoT=np.ascontiguousarray(WoT2),
        W1T=np.ascontiguousarray(W1.T).astype(np.float16),
        W2T=np.ascontiguousarray(W2.T).astype(np.float16),
    )

    pfr = pf.reshape(K, T, PD)
    tgrid = np.arange(128)[:, None] + 128 * np.arange(NCH)[None, :]   # [128, NCH]

    in_maps = []
    for core in range(NCORES):
        ig = np.arange(core * IPC, (core + 1) * IPC)
        mlt = (tgrid[:, :, None] < ig[None, None, :]).astype(np.float32)
        mge = ((tgrid[:, :, None] >= ig[None, None, :])
               & (tgrid[:, :, None] <= T - 1)).astype(np.float16)
        # [chunk, t, i, pd] layout -> each tile DMA is one contiguous burst
        shard = np.zeros((NCH * 128, IPC, PD), np.float16)
        shard[:T] = (pfr[ig] * colscale[None, None, :]).transpose(1, 0, 2)
        xi = x[ig]
        cb32 = b32a.copy()
        cb32[:, 640:896] = mlt.reshape(128, NCH * IPC)
        cb16 = b16a.copy()
        cb16[:, 1024:1280] = mge.reshape(128, NCH * IPC)
        cb16[0:IPC, 1288:1800] = xi.astype(np.float16)
        m = dict(base)
        m.update(
            pf=shard.reshape(NCH, 128, IPC, PD),
            xi=xi.astype(np.float32),
            b32=cb32,
            b16=cb16,
        )
        in_maps.append(m)
    return in_maps


_COMPILED = None


def _get_program() -> bacc.Bacc:
    global _COMPILED
    if _COMPILED is None:
        nc = build_program()
        nc.compile()
        _COMPILED = nc
    return _COMPILED


TRACE = False
LAST_RESULT = None


def _install_axon_ntff_hook():
    """The container's antenv lacks axon_hooks; recreate it from trn_boot's
    ctypes implementation so trace=True can capture NTFF profiles."""
    import sys
    import types
    try:
        from antenv.axon_hooks import get_axon_ntff_profile_hook  # noqa: F401
        return
    except ImportError:
        pass
    from trn_agent_boot.trn_boot import _ntff_profile_via_ctypes
    hook = _ntff_profile_via_ctypes("/opt/axon/libaxon_pjrt.so")
    m = types.ModuleType("antenv.axon_hooks")
    m.get_axon_ntff_profile_hook = lambda: hook
    sys.modules["antenv.axon_hooks"] = m


def kernel(**inputs) -> np.ndarray:
    import concourse.bass_utils as bu
    from concourse.bass_utils import run_bass_kernel_spmd
    global LAST_RESULT
    if TRACE:
        _install_axon_ntff_hook()
        bu.upload_artifacts = lambda tmpdir: str(tmpdir)  # no bucket here
    nc = _get_program()
    in_maps = prep_in_maps(inputs)
    res = run_bass_kernel_spmd(nc, in_maps, list(range(NCORES)), trace=TRACE)
    LAST_RESULT = res
    outs = [res.results[c]["out"] for c in range(NCORES)]
    return np.concatenate(outs, axis=0).astype(np.float32)


# revision 49
# speedup vs baseline: 1.0037x; 1.0020x over previous
"""Trainium2 Bass kernel for nn_ARTLayer (gnn_message_passing).

Math (reference):
    j(i,t) = t + (t>=i)                                    # [K, K-1] neighbor index
    alpha  = sigmoid(x@wa [i] + x@wb [j] + pf@wc + b_att)  # [K, K-1]
    msgs   = mean_t alpha * ((x@WobjT + b_obj)[j] + pf@WpairT + b_pair)
    out    = LN(x + msgs); out = LN(out + FFN(out))

Key algebraic rewrite (removes the 34-GFLOP [P,PD]x[PD,H] einsum):
    sum_t a*(pf@WpT)  = (sum_t a*pf) @ WpT               -> U[i,:] @ WpT
    sum_t a*oj[j]     = (A @ x) @ WobjT                  -> Gx[i,:] @ WoT
      with A[i,j] decomposed via lo/hi shifted views of x and a t>=i mask
    sum_t a*(b_obj+b_pair) = s_alpha[i] * bop

Sharding: rows i split across 8 cores (64 each); small tensors replicated;
host concatenates the per-core [64, 512] outputs.

Implementation notes (driven by NTFF profiles):
  - wc is folded into pf on the host (column scales, floored at fp16
    min-normal); sc becomes a pure fp16 halving-tree add-reduce and U is
    recovered exactly via W_pairT rows pre-divided by the scales.
  - 1/511 (the neighbor mean) is folded into WpT/WoT/bop on the host, and
    the t=511 pad slot is poisoned with -1e9 pre-sigmoid, so raw sigmoid
    output is used directly with no mask/scale multiplies.
  - U accumulation runs as M=4 quad matmuls (alpha quad stationary, four pf
    blocks streaming at N=512); the wanted rows sit on the block diagonal
    and are gathered by a stride-640 DRAM access pattern after a bank dump.
  - pf is re-laid-out on the host to [chunk, t, i, pd] so each tile DMA is
    one fully-contiguous 16KB-per-partition burst. One HWDGE queue (sync)
    carries the critical path in order (packed const blobs, then pf chunks);
    tail-only weights stream on the second HWDGE queue (scalar) in parallel.
  - U rows are extracted on-chip (bank copy -> per-block PE transpose ->
    free-strided gather copies); no DRAM bounce.
"""
import numpy as np

import concourse.bass as bass
import concourse.tile as tile
from concourse import bacc, mybir

F32, F16 = mybir.dt.float32, mybir.dt.float16
AX = mybir.AxisListType
OP = mybir.AluOpType
AF = mybir.ActivationFunctionType

K, D, H, PD = 512, 512, 512, 128
T = K - 1                      # 511 neighbors per row
NCORES, IPC = 8, 64            # rows per core
NCH = 4                        # t-chunks of 128 (last chunk row 127 is t=511 pad)
IB, NIB = 64, 1                # i-block within a core
EPS = 1e-5


def build_program() -> bacc.Bacc:
    nc = bacc.Bacc("TRN2", target_bir_lowering=False, debug=False)

    def inp(name, shape, dt):
        return nc.dram_tensor(name, shape, dt, kind="ExternalInput").ap()

    pf = inp("pf", [NCH, 128, IPC, PD], F16)    # [chunk, t-in-chunk, i, pd]
    # single-DMA packed critical constants (128 contiguous rows each):
    # b32: [cmat 0:512 | poison 512:640 | mask_lt 640:896 | b_att 896:904]
    # b16: [wa_t 0:512 | wb_t 512:1024 | mask_ge 1024:1280 | ones 1280:1288
    #       | xi16 1288:1800]
    b32 = inp("b32", [128, 904], F32)
    b16 = inp("b16", [128, 1800], F16)
    xlo_ch = inp("xlo_ch", [128, NCH, D], F16)  # x rows chunked [t%128, t//128]
    dxf = inp("dxf", [K, D], F16)               # x[t+1] - x[t], host computed
    xi = inp("xi", [IPC, D], F32)               # this core's rows of x
    bias5 = inp("bias5", [5, H], F32)           # [ln_g; ln_b; b1; b2; bop/511]
    WpT = inp("WpT", [PD, H], F16)              # W_pair.T / colscale / 511
    WoT = inp("WoT", [D, H], F16)               # W_obj.T / 511
    W1T = inp("W1T", [H, H], F16)
    W2T = inp("W2T", [H, H], F16)

    out_d = nc.dram_tensor("out", [IPC, H], F32, kind="ExternalOutput").ap()

    with tile.TileContext(nc) as tc:
        with (
            tc.tile_pool(name="const", bufs=1) as cpool,
            tc.tile_pool(name="pfp", bufs=4) as pfp,
            tc.tile_pool(name="scrp", bufs=3) as scrp,
            tc.tile_pool(name="smallp", bufs=4) as smallp,
            tc.tile_pool(name="postp", bufs=3) as postp,
            tc.tile_pool(name="pss", bufs=2, space="PSUM") as pss,
            tc.tile_pool(name="psflex", bufs=4, space="PSUM") as psflex,
            tc.tile_pool(name="psmp", bufs=1, space="PSUM") as psmp,
            tc.tile_pool(name="psgp", bufs=1, space="PSUM") as psgp,
        ):
            # DMA policy: one HWDGE queue (sync) carries the critical path
            # in program order (a single queue reaches ~313 GB/s here and
            # multi-queue round-robin measures WORSE); bulky tail-only
            # weights stream on the gpsimd SWDGE queue in parallel.
            def dma(out, in_):
                nc.sync.dma_start(out=out, in_=in_)

            def dma_late(out, in_):
                nc.scalar.dma_start(out=out, in_=in_)

            # ---- constants & weights to SBUF (3 packed critical DMAs) ----
            hp = tc.high_priority()
            hp.__enter__()
            b32_sb = cpool.tile([128, 904], F32)
            dma(b32_sb, b32)
            b16_sb = cpool.tile([128, 1800], F16)
            dma(b16_sb, b16)
            xlo = cpool.tile([128, NCH, D], F16)
            dma(xlo, xlo_ch)
            hp.__exit__(None, None, None)
            tc.no_sync_barrier()   # keep pf DMAs behind the critical consts
            id_sb = b32_sb[:, 0:128]
            ones_sb = b32_sb[:, 128:256]
            sh1_sb = b32_sb[:, 256:384]
            sh2_sb = b32_sb[:, 384:512]
            poison_sb = b32_sb[0:1, 512:640]
            mlt_sb = b32_sb[:, 640:896].rearrange("p (c i) -> p c i", c=NCH)
            b_att_col = b32_sb[0:IPC, 896:897]
            wa_b = b16_sb[0:IPC, 0:512]
            wb_b = b16_sb[:, 512:1024]
            mge_sb = b16_sb[:, 1024:1280].rearrange("p (c i) -> p c i", c=NCH)
            ones16_sb = b16_sb[:, 1280:1288]
            xi16_sb = b16_sb[0:IPC, 1288:1800]
            bias_sb = cpool.tile([IPC, 5, H], F32)
            dma_late(bias_sb, bias5[None, :, :].to_broadcast([IPC, 5, H]))
            gb_sb = bias_sb[:, 0, :]
            bb_sb = bias_sb[:, 1, :]
            b1p_row = bias_sb[0:1, 2, :]
            lb2_sb = bias_sb[:, 3, :]
            bop_row = bias_sb[0:1, 4, :]
            # tail-only loads on the slow queue, in rough use order
            dx = cpool.tile([128, NCH, D], F16)
            dma_late(dx, dxf.rearrange("(c p) d -> p c d", p=128))
            WpT_sb = cpool.tile([128, H], F16)
            dma_late(WpT_sb, WpT)
            WoT_sb = cpool.tile([128, NCH, H], F16)
            dma_late(WoT_sb, WoT.rearrange("(c p) h -> p c h", p=128))
            xi_sb = cpool.tile([IPC, D], F32)
            dma_late(xi_sb, xi)
            W1T_sb = cpool.tile([128, NCH, H], F16)
            dma_late(W1T_sb, W1T.rearrange("(c p) h -> p c h", p=128))
            W2T_sb = cpool.tile([128, NCH, H], F16)
            dma_late(W2T_sb, W2T.rearrange("(c p) h -> p c h", p=128))

            eps_col = cpool.tile([IPC, 1], F32)
            nc.vector.memset(eps_col, EPS)

            # ---- sa (this core's rows) and sb (all rows) ----
            scr_sa = smallp.tile([IPC, D], F16)
            nc.vector.tensor_mul(scr_sa, xi16_sb, wa_b)
            sa_col = smallp.tile([IPC, 1], F32)
            nc.vector.tensor_reduce(sa_col, scr_sa, axis=AX.X, op=OP.add)
            nc.vector.tensor_add(sa_col, sa_col, b_att_col)
            sa_diag = smallp.tile([IPC, IPC], F32)
            nc.vector.tensor_mul(sa_diag, id_sb[0:IPC, 0:IPC],
                                 sa_col.to_broadcast([IPC, IPC]))

            sb_cols = smallp.tile([128, NCH], F32)
            for c in range(NCH):
                scr_sb = smallp.tile([128, D], F16)
                nc.vector.tensor_mul(scr_sb, xlo[:, c, :], wb_b)
                nc.vector.tensor_reduce(
                    sb_cols[:, c:c + 1], scr_sb, axis=AX.X, op=OP.add)

            # sb_hi[p, c] = sb[c*128+p+1] via shift matmuls; slot 511 stays 0
            sbhi_ps = pss.tile([128, NCH], F32, tag="ps_small")
            nc.tensor.matmul(sbhi_ps, sh1_sb, sb_cols, start=True, stop=False)
            nc.tensor.matmul(sbhi_ps[:, 0:NCH - 1], sh2_sb, sb_cols[:, 1:NCH],
                             start=False, stop=True)
            sbhi_cols = smallp.tile([128, NCH], F32)
            nc.vector.tensor_copy(sbhi_cols, sbhi_ps)

            # ---- SBJ[t, i] = sa[i] + b_att + sb_hi[t] + mask_lt*(sb_lo-sb_hi),
            #      with -1e9 poison at the t=511 pad slot ----
            sbj = cpool.tile([128, NCH, IPC], F32)
            for c in range(NCH):
                diffc = smallp.tile([128, 1], F32)
                nc.vector.tensor_tensor(
                    diffc, sb_cols[:, c:c + 1], sbhi_cols[:, c:c + 1], OP.subtract)
                diagc = smallp.tile([128, 128], F32)
                nc.vector.tensor_mul(diagc, id_sb, diffc.to_broadcast([128, 128]))
                diagb = smallp.tile([128, 128], F32)
                nc.vector.tensor_mul(
                    diagb, id_sb, sbhi_cols[:, c:c + 1].to_broadcast([128, 128]))
                ps_sbj = pss.tile([128, IPC], F32, tag="ps_small")
                nc.tensor.matmul(ps_sbj, ones_sb[0:IPC, :], sa_diag,
                                 start=True, stop=False)
                nc.tensor.matmul(ps_sbj, diagb, ones_sb[:, 0:IPC],
                                 start=False, stop=False)
                if c == NCH - 1:
                    # poison: sigmoid(-1e9) = 0 exactly, pad row drops out
                    nc.tensor.matmul(ps_sbj, poison_sb, ones_sb[0:1, 0:IPC],
                                     start=False, stop=False)
                nc.tensor.matmul(ps_sbj, diagc, mlt_sb[:, c, :],
                                 start=False, stop=True)
                nc.vector.tensor_copy(sbj[:, c, :], ps_sbj)

            # ---- main edge pass ----
            alpha_full = cpool.tile([128, NCH, IPC], F16)   # raw sigmoid out
            age_full = cpool.tile([128, NCH, IPC], F16)     # masked (t>=i) alpha
            gx_ps = psgp.tile([IPC, D], F32)                # sum_t a*x[j]
            msg_ps = psmp.tile([IPC, H], F32)
            s_ps = pss.tile([1, IPC], F32, tag="ps_small")
            # U quad rows: bank b, partition slot 32s..32s+3 holds i=16b+4s+j
            u_ps = [psflex.tile([128, 512], F32, tag="flex", name=f"u_ps{b}")
                    for b in range(4)]
            for b in range(4):
                nc.vector.memset(u_ps[b], 0.0)

            for c in range(NCH):
                pf_t = pfp.tile([128, IB, PD], F16, tag="pf_t")
                dma(pf_t, pf[c, :, :, :])
                # sc = sum_pd pf_sent (wc pre-folded): fp16 halving tree
                scr = scrp.tile([128, IB, 64], F16, tag="scr")
                nc.vector.tensor_add(scr, pf_t[:, :, 0:64], pf_t[:, :, 64:128])
                w = 32
                while w >= 2:
                    nc.vector.tensor_add(
                        scr[:, :, 0:w], scr[:, :, 0:w], scr[:, :, w:2 * w])
                    w //= 2
                sc_t = smallp.tile([128, IB], F32, tag="sc_t")
                nc.vector.tensor_add(sc_t, scr[:, :, 0], scr[:, :, 1])
                aarg = smallp.tile([128, IB], F32)
                nc.vector.tensor_add(aarg, sc_t, sbj[:, c, :])
                nc.scalar.activation(alpha_full[:, c, :], aarg, AF.Sigmoid)
                nc.vector.tensor_mul(age_full[:, c, :], alpha_full[:, c, :],
                                     mge_sb[:, c, :])
                # U quads: lhsT = 4 alpha columns, rhs = 4 pf blocks; the
                # wanted rows sit on the diagonal (gathered via DRAM AP)
                for q in range(IB // 4):
                    b, sp = divmod(q, 4)
                    nc.tensor.matmul(
                        u_ps[b][32 * sp:32 * sp + 4, :],
                        alpha_full[:, c, 4 * q:4 * q + 4],
                        pf_t[:, 4 * q:4 * q + 4, :],
                        start=(c == 0), stop=(c == NCH - 1),
                        tile_position=(0, 32 * sp))
                nc.tensor.matmul(gx_ps, alpha_full[:, c, :], xlo[:, c, :],
                                 start=(c == 0), stop=False)
                nc.tensor.matmul(s_ps, ones16_sb[:, 0:1], alpha_full[:, c, :],
                                 start=(c == 0), stop=(c == NCH - 1))

            # scheduler fence: keep every tail instruction after the loop in
            # each engine stream (strict-FIFO engines head-of-line block if
            # e.g. an LN Sqrt lands between loop sigmoids in the ACT queue)
            tc.no_sync_barrier()

            # G2 (shifted-x correction) after the loop: dx arrives on the slow
            # queue and age_full persists, so this overlaps the loop tail
            for c in range(NCH):
                nc.tensor.matmul(gx_ps, age_full[:, c, :], dx[:, c, :],
                                 start=False, stop=(c == NCH - 1))

            # ---- messages = U@WpT + Gx@WoT + s_alpha x bop ----
            s_row = smallp.tile([1, IPC], F32)
            nc.vector.tensor_copy(s_row, s_ps)

            # U reassembly on-chip: bank copy -> PE transpose of each
            # 128-col block (diagonal quad becomes free-strided columns) ->
            # tiny strided copies assemble UT directly; no DRAM bounce.
            u_sb = postp.tile([128, IPC], F16)
            for b in range(4):
                u_cp = postp.tile([128, 512], F32, tag="u_cp")
                nc.vector.tensor_copy(u_cp, u_ps[b])
                for j in range(4):
                    ptu = pss.tile([128, 128], F32, tag="ps_small")
                    nc.tensor.transpose(ptu, u_cp[:, j * 128:(j + 1) * 128],
                                        id_sb)
                    # cols {j, 32+j, 64+j, 96+j} hold U rows i=16b+4s+j
                    src_ap = ptu.rearrange("p (s q) -> p s q", q=32)[:, :, j]
                    dst_ap = u_sb.rearrange("p (r s f) -> p r s f", r=4, s=4)[
                        :, b, :, j]
                    nc.vector.tensor_copy(dst_ap, src_ap)

            gx_sb = postp.tile([IPC, D], F32)
            nc.vector.tensor_copy(gx_sb, gx_ps)
            gxT = postp.tile([128, NCH, IPC], F16)
            for c in range(NCH):
                ptg = pss.tile([128, IPC], F32, tag="ps_small")
                nc.tensor.transpose(ptg, gx_sb[:, c * 128:(c + 1) * 128],
                                    id_sb[0:IPC, 0:IPC])
                nc.vector.tensor_copy(gxT[:, c, :], ptg)

            # gx-side MMs carry the accumulation-group start (their input is
            # ready first); Upp closes the group after the U extraction
            for c in range(NCH):
                nc.tensor.matmul(msg_ps, gxT[:, c, :], WoT_sb[:, c, :],
                                 start=(c == 0), stop=False)
            nc.tensor.matmul(msg_ps, s_row, bop_row, start=False, stop=False)
            nc.tensor.matmul(msg_ps, u_sb, WpT_sb, start=False, stop=True)

            # ---- residual + LN1 ----
            def layer_norm(v, bare=False):
                stats = smallp.tile([IPC, 6], F32)
                nc.vector.bn_stats(out=stats, in_=v)
                mv = smallp.tile([IPC, 2], F32)
                nc.vector.bn_aggr(out=mv, in_=stats)
                std = smallp.tile([IPC, 1], F32)
                nc.scalar.activation(std, mv[:, 1:2], AF.Sqrt, bias=eps_col)
                rstd = smallp.tile([IPC, 1], F32)
                nc.vector.reciprocal(rstd, std)
                cen = postp.tile([IPC, H], F32)
                nc.vector.tensor_scalar(cen, v, mv[:, 0:1], rstd,
                                        OP.subtract, OP.mult)
                if bare:
                    return cen     # ln_g/ln_b folded into W1T'/b1' on host
                o = postp.tile([IPC, H], F32)
                nc.vector.tensor_mul(o, cen, gb_sb)
                nc.vector.tensor_add(o, o, bb_sb)
                return o

            h_sb = postp.tile([IPC, H], F32)
            nc.vector.tensor_add(h_sb, xi_sb, msg_ps)
            out1 = layer_norm(h_sb, bare=True)

            # ---- FFN ----
            def transpose_rows(v):
                vT = postp.tile([128, NCH, IPC], F16, tag="vT")
                for c in range(NCH):
                    ptt = pss.tile([128, IPC], F32, tag="ps_small")
                    nc.tensor.transpose(ptt, v[:, c * 128:(c + 1) * 128],
                                        id_sb[0:IPC, 0:IPC])
                    nc.vector.tensor_copy(vT[:, c, :], ptt)
                return vT

            o1T = transpose_rows(out1)
            # true out1 + b2 rebuilt off the critical path (gb, lnb+b2 rows)
            o1b = postp.tile([IPC, H], F32)
            nc.vector.tensor_mul(o1b, out1, gb_sb)
            nc.vector.tensor_add(o1b, o1b, lb2_sb)
            f1_ps = psflex.tile([IPC, H], F32, tag="flex")
            for c in range(NCH):
                nc.tensor.matmul(f1_ps, o1T[:, c, :], W1T_sb[:, c, :],
                                 start=(c == 0), stop=False)
            # b1' bias lands in PSUM as a rank-1 matmul (keeps DVE off-path)
            nc.tensor.matmul(f1_ps, ones_sb[0:1, 0:IPC], b1p_row,
                             start=False, stop=True)
            f1 = postp.tile([IPC, H], F32)
            nc.vector.tensor_scalar_max(f1, f1_ps, 0.0)

            f1T = transpose_rows(f1)
            f2_ps = psflex.tile([IPC, H], F32, tag="flex")
            for c in range(NCH):
                nc.tensor.matmul(f2_ps, f1T[:, c, :], W2T_sb[:, c, :],
                                 start=(c == 0), stop=(c == NCH - 1))
            h2 = postp.tile([IPC, H], F32)
            nc.vector.tensor_add(h2, f2_ps, o1b)
            out2 = layer_norm(h2)


            nc.sync.dma_start(out=out_d, in_=out2)

    return nc


def _poison() -> np.ndarray:
    p = np.zeros((1, 128), np.float32)
    p[0, 127] = -1e9
    return p


def _cmat() -> np.ndarray:
    c = np.zeros((128, 4, 128), np.float32)
    c[:, 0, :] = np.eye(128)
    c[:, 1, :] = 1.0
    c[:, 2, :] = np.eye(128, k=-1)     # shift1[q, p] = (q == p+1)
    c[0, 3, 127] = 1.0                  # shift2[q, p] = (q==0)&(p==127)
    return c


def prep_in_maps(inputs) -> list[dict]:
    x = np.asarray(inputs["x"], np.float32)
    pf = np.asarray(inputs["pair_feats"], np.float32)
    W_att = np.asarray(inputs["W_att"], np.float32)
    b_att = np.asarray(inputs["b_att"], np.float32)
    W_obj = np.asarray(inputs["W_obj"], np.float32)
    b_obj = np.asarray(inputs["b_obj"], np.float32)
    W_pair = np.asarray(inputs["W_pair"], np.float32)
    b_pair = np.asarray(inputs["b_pair"], np.float32)
    ln_g = np.asarray(inputs["ln_g"], np.float32)
    ln_b = np.asarray(inputs["ln_b"], np.float32)
    W1 = np.asarray(inputs["W1"], np.float32)
    b1 = np.asarray(inputs["b1"], np.float32)
    W2 = np.asarray(inputs["W2"], np.float32)
    b2 = np.asarray(inputs["b2"], np.float32)

    wa, wb, wc = W_att[0, :D], W_att[0, D:2 * D], W_att[0, 2 * D:]
    xpad = np.concatenate([x, np.zeros((1, D), np.float32)], axis=0)

    # fold wc into pf columns; recover U via pre-divided W_pair.T rows.
    colscale = np.sign(wc) * np.maximum(np.abs(wc), 6e-5)
    colscale[colscale == 0] = 6e-5
    # 1/511 (the mean over neighbors) is folded into the three weight paths
    # that consume raw alpha: U@WpT, (A@x)@WoT, and s_alpha*bop.
    WpT2 = (W_pair.T / colscale[:, None] / T).astype(np.float16)
    WoT2 = (W_obj.T / T).astype(np.float16)
    dxf = np.diff(xpad[:K + 1], axis=0)

    b32a = np.zeros((128, 904), np.float32)
    b32a[:, 0:512] = _cmat().reshape(128, 512)
    b32a[0, 512 + 127] = -1e9
    b32a[:, 896] = b_att[0]
    b16a = np.zeros((128, 1800), np.float16)
    b16a[:, 0:512] = wa[None, :]
    b16a[:, 512:1024] = wb[None, :]
    b16a[:, 1280:1288] = 1.0
    xlo_np = np.ascontiguousarray(
        x.reshape(NCH, 128, D).transpose(1, 0, 2)).astype(np.float16)

    base = dict(
        xlo_ch=xlo_np,
        dxf=dxf.astype(np.float16),
        bias5=np.stack([ln_g, ln_b, b1 + ln_b @ W1.T, ln_b + b2,
                        (b_obj + b_pair) / T]).astype(np.float32),
        WpT=np.ascontiguousarray(WpT2),
        WoT=np.ascontiguousarray(WoT2),
        W1T=np.ascontiguousarray(W1.T * ln_g[:, None]).astype(np.float16),
        W2T=np.ascontiguousarray(W2.T).astype(np.float16),
    )

    pfr = pf.reshape(K, T, PD)
    tgrid = np.arange(128)[:, None] + 128 * np.arange(NCH)[None, :]   # [128, NCH]

    in_maps = []
    for core in range(NCORES):
        ig = np.arange(core * IPC, (core + 1) * IPC)
        mlt = (tgrid[:, :, None] < ig[None, None, :]).astype(np.float32)
        mge = ((tgrid[:, :, None] >= ig[None, None, :])
               & (tgrid[:, :, None] <= T - 1)).astype(np.float16)
        # [chunk, t, i, pd] layout -> each tile DMA is one contiguous burst
        shard = np.zeros((NCH * 128, IPC, PD), np.float16)
        shard[:T] = (pfr[ig] * colscale[None, None, :]).transpose(1, 0, 2)
        xi = x[ig]
        cb32 = b32a.copy()
        cb32[:, 640:896] = mlt.reshape(128, NCH * IPC)
        cb16 = b16a.copy()
        cb16[:, 1024:1280] = mge.reshape(128, NCH * IPC)
        cb16[0:IPC, 1288:1800] = xi.astype(np.float16)
        m = dict(base)
        m.update(
            pf=shard.reshape(NCH, 128, IPC, PD),
            xi=xi.astype(np.float32),
            b32=cb32,
            b16=cb16,
        )
        in_maps.append(m)
    return in_maps


_COMPILED = None


def _get_program() -> bacc.Bacc:
    global _COMPILED
    if _COMPILED is None:
        nc = build_program()
        nc.compile()
        _COMPILED = nc
    return _COMPILED


TRACE = False
LAST_RESULT = None


def _install_axon_ntff_hook():
    """The container's antenv lacks axon_hooks; recreate it from trn_boot's
    ctypes implementation so trace=True can capture NTFF profiles."""
    import sys
    import types
    try:
        from antenv.axon_hooks import get_axon_ntff_profile_hook  # noqa: F401
        return
    except ImportError:
        pass
    from trn_agent_boot.trn_boot import _ntff_profile_via_ctypes
    hook = _ntff_profile_via_ctypes("/opt/axon/libaxon_pjrt.so")
    m = types.ModuleType("antenv.axon_hooks")
    m.get_axon_ntff_profile_hook = lambda: hook
    sys.modules["antenv.axon_hooks"] = m


def kernel(**inputs) -> np.ndarray:
    import concourse.bass_utils as bu
    from concourse.bass_utils import run_bass_kernel_spmd
    global LAST_RESULT
    if TRACE:
        _install_axon_ntff_hook()
        bu.upload_artifacts = lambda tmpdir: str(tmpdir)  # no bucket here
    nc = _get_program()
    in_maps = prep_in_maps(inputs)
    res = run_bass_kernel_spmd(nc, in_maps, list(range(NCORES)), trace=TRACE)
    LAST_RESULT = res
    outs = [res.results[c]["out"] for c in range(NCORES)]
    return np.concatenate(outs, axis=0).astype(np.float32)
